# revision 13
# baseline (speedup 1.0000x reference)
"""Bass/Tile TRN2 kernel for nn_Attention (Bahdanau-style attention scores).

Computation (per batch b):
    energy[s, h] = tanh( (enc[b] @ We)[s, h] + (hidden[b] @ Wh)[h] + bias[h] )
    scores[s]    = sum_h energy[s, h] * v[h]
    out[b]       = softmax(scores)

Sharding: data-parallel over batch B=32 across 8 cores (4 batches/core);
W, b, v replicated.

Per-core device program (all matmuls on PE):
  - enc tiles are PE-transposed (fp32, exact) to get the contraction dim (e)
    onto partitions.
  - main matmul We.T-tile @ encT in float32r (TF32-like, 1 cyc/row at N=512,
    fp32 PSUM accumulate), output layout [h, s] so the (h@Wh + b) bias is a
    per-partition scalar fused into the ScalarE tanh.
  - v-dot as a k=h matmul with v as a [128,1] stationary.
  - softmax over s on partition 0 (reduce_max -> exp with fused sum -> mul).
"""

import os
import numpy as np

import concourse.bass as bass
import concourse.tile as tile
from concourse import bacc, mybir
from concourse import bass_utils
from concourse.masks import make_identity

F32 = mybir.dt.float32
F32R = mybir.dt.float32r
BF16 = mybir.dt.bfloat16
AFT = mybir.ActivationFunctionType
AXX = mybir.AxisListType.X

N_CORES = 8
B = 32
B_LOC = B // N_CORES  # 4
S = 1024
H = 512
E2 = 2 * H  # 1024
P = 128
N_HT = H // P   # 4 h-tiles
N_ET = E2 // P  # 8 e-tiles
N_SC = S // 512  # 2 s-chunks of 512


USE_BF16 = False


def build(use_bf16=None):
    if use_bf16 is None:
        use_bf16 = USE_BF16
    nc = bacc.Bacc("TRN2", target_bir_lowering=False, debug=False)
    hidden = nc.dram_tensor("hidden", [B_LOC, H], F32, kind="ExternalInput").ap()
    enc = nc.dram_tensor("enc", [B_LOC, S, E2], F32, kind="ExternalInput").ap()
    W = nc.dram_tensor("W", [3 * H, H], F32, kind="ExternalInput").ap()
    bvec = nc.dram_tensor("b", [H], F32, kind="ExternalInput").ap()
    vvec = nc.dram_tensor("v", [H], F32, kind="ExternalInput").ap()
    out = nc.dram_tensor("out", [B_LOC, S], F32, kind="ExternalOutput").ap()

    with tile.TileContext(nc) as tc:
        with (
            tc.tile_pool(name="consts", bufs=1) as consts,
            tc.tile_pool(name="encp", bufs=8) as encp,
            tc.tile_pool(name="encTp", bufs=12) as encTp,
            tc.tile_pool(name="enccp", bufs=6) as enccp,
            tc.tile_pool(name="energyp", bufs=6) as energyp,
            tc.tile_pool(name="smp", bufs=2) as smp,
            tc.tile_pool(name="tpps", bufs=3, space="PSUM") as tpps,
            tc.tile_pool(name="outps", bufs=4, space="PSUM") as outps,
            tc.tile_pool(name="scps", bufs=1, space="PSUM") as scps,
        ):
            # ---- prefetch first s-chunk of enc on the sync (HWDGE) queue ----
            first_enc = []
            for st in range(4):
                t0 = encp.tile([P, E2], F32, tag="enc", name=f"enc_pre{st}")
                nc.sync.dma_start(t0[:], enc[0, st * P:(st + 1) * P, :])
                first_enc.append(t0)

            # ---- constants: weight/vector DMAs on the gpsimd (SWDGE) queue ----
            ident = consts.tile([P, P], F32)
            make_identity(nc, ident[:])
            cast_dt = BF16 if use_bf16 else F32R
            ident_c = consts.tile([P, P], cast_dt)
            nc.vector.tensor_copy(ident_c[:], ident[:])

            We_sb = consts.tile([P, N_ET, H], F32)  # [e_in_tile, e_tile, h]
            We_r = consts.tile([P, N_ET, H], cast_dt)
            for j in range(N_ET):
                nc.gpsimd.dma_start(
                    We_sb[:, j, :], W[H + j * P:H + (j + 1) * P, :]
                )
                nc.vector.tensor_copy(We_r[:, j, :], We_sb[:, j, :])
            Wh_sb = consts.tile([P, N_HT, H], F32)
            nc.gpsimd.dma_start(
                Wh_sb[:], W[:H, :].rearrange("(j e) h -> e j h", e=P)
            )
            Wh_r = consts.tile([P, N_HT, H], F32R)
            nc.vector.tensor_copy(Wh_r[:], Wh_sb[:])
            v_sb = consts.tile([P, N_HT], F32)  # column i = v[i*128:(i+1)*128]
            nc.gpsimd.dma_start(v_sb[:], vvec.rearrange("(i e) -> e i", e=P))
            v_r = consts.tile([P, N_HT], F32R)
            nc.vector.tensor_copy(v_r[:], v_sb[:])
            b_sb = consts.tile([P, N_HT], F32)
            nc.gpsimd.dma_start(b_sb[:], bvec.rearrange("(i e) -> e i", e=P))
            hT_sb = consts.tile([P, N_HT, B_LOC], F32)  # hidden^T tiles [e, b]
            for t in range(N_HT):
                nc.gpsimd.dma_start(
                    hT_sb[:, t, :],
                    hidden[:, t * P:(t + 1) * P].rearrange("b e -> e b"),
                )
            hT_r = consts.tile([P, N_HT, B_LOC], F32R)
            nc.vector.tensor_copy(hT_r[:], hT_sb[:])

            bias_sb = consts.tile([P, N_HT, B_LOC], F32)

            def emit_bias_setup():
                # hproj as [b, h] wide-N matmul, then PE-transpose to [h, b];
                # bias[h, b] = hproj[h, b] + b[h]
                ps_h = tpps.tile([B_LOC, H], F32, tag="tstage", name="ps_h")
                for j in range(N_HT):
                    nc.tensor.matmul(
                        ps_h[:],
                        hT_r[:, j, :],
                        Wh_r[:, j, :],
                        start=(j == 0),
                        stop=(j == N_HT - 1),
                    )
                hp_sb = consts.tile([B_LOC, H], F32, name="hp_sb")
                nc.vector.tensor_copy(hp_sb[:], ps_h[:])
                for i in range(N_HT):
                    tp_i = tpps.tile([P, B_LOC], F32, tag="tstage", name=f"tp_i{i}")
                    nc.tensor.transpose(
                        tp_i[:], hp_sb[:, i * P:(i + 1) * P], ident[:B_LOC, :B_LOC]
                    )
                    nc.vector.tensor_scalar_add(
                        bias_sb[:, i, :], tp_i[:], b_sb[:, i:i + 1]
                    )

            # ---- main loop ----
            for bi in range(B_LOC):
                scores_sb = smp.tile([1, S], F32, tag="scores")
                for sc in range(N_SC):
                    s0 = sc * 512
                    if bi == 0 and sc == 0:
                        enc_tiles = first_enc
                    else:
                        enc_tiles = []
                        for st in range(4):
                            t = encp.tile([P, E2], F32, tag="enc")
                            nc.sync.dma_start(
                                t[:], enc[bi, s0 + st * P: s0 + (st + 1) * P, :]
                            )
                            enc_tiles.append(t)

                    psum_out = [
                        outps.tile([P, 512], F32, tag="mmout", name=f"mmout{i}")
                        for i in range(N_HT)
                    ]

                    enc_c = []
                    for st in range(4):
                        ec = enccp.tile([P, E2], cast_dt, tag="encc", name=f"encc{st}")
                        nc.vector.tensor_copy(ec[:], enc_tiles[st][:])
                        enc_c.append(ec)

                    encT = [None] * N_ET

                    def emit_transpose(j):
                        tp = tpps.tile([P, 512], cast_dt, tag="tstage", name=f"tp{j}")
                        for st in range(4):
                            nc.tensor.transpose(
                                tp[:, st * P:(st + 1) * P],
                                enc_c[st][:, j * P:(j + 1) * P],
                                ident_c[:],
                            )
                        e = encTp.tile([P, 512], cast_dt, tag="encT", name=f"encT{j}")
                        nc.vector.tensor_copy(e[:], tp[:])
                        encT[j] = e

                    def emit_matmuls(j):
                        for i in range(N_HT):
                            nc.tensor.matmul(
                                psum_out[i][:],
                                We_r[:, j, i * P:(i + 1) * P],
                                encT[j][:],
                                start=(j == 0),
                                stop=(j == N_ET - 1),
                            )

                    # software-pipelined emission: transposes run 2 e-slices
                    # ahead of the matmuls so the PE never waits on the DVE
                    # PSUM->SBUF copy.
                    if bi == 0 and sc == 0:
                        for j in range(N_ET):
                            emit_transpose(j)
                        for j in range(N_ET):
                            emit_matmuls(j)
                    else:
                        emit_transpose(0)
                        emit_transpose(1)
                        for j in range(N_ET):
                            if j + 2 < N_ET:
                                emit_transpose(j + 2)
                            emit_matmuls(j)

                    if bi == 0 and sc == 0:
                        # placed here so the slow weight-DMA -> cast chain it
                        # depends on never blocks the chunk-0 PE work (the PE
                        # executes strictly in program order).
                        emit_bias_setup()

                    sc_ps = scps.tile([1, 512], F32, tag="scores_ps")
                    for i in range(N_HT):
                        en = energyp.tile([P, 512], F32R, tag="energy", name=f"en{i}")
                        nc.scalar.activation(
                            en[:],
                            psum_out[i][:],
                            AFT.Tanh,
                            bias=bias_sb[:, i, bi:bi + 1],
                        )
                        nc.tensor.matmul(
                            sc_ps[:],
                            v_r[:, i:i + 1],
                            en[:],
                            start=(i == 0),
                            stop=(i == N_HT - 1),
                        )
                    nc.vector.tensor_copy(scores_sb[:, s0:s0 + 512], sc_ps[:])

                # ---- softmax over s (partition 0) ----
                negmax = smp.tile([1, 1], F32, tag="negmax")
                nc.vector.reduce_max(
                    out=negmax[:], in_=scores_sb[:], axis=AXX, negate=True
                )
                exp_sb = smp.tile([1, S], F32, tag="exp")
                ssum = smp.tile([1, 1], F32, tag="ssum")
                nc.scalar.activation(
                    exp_sb[:], scores_sb[:], AFT.Exp, bias=negmax[:], accum_out=ssum[:]
                )
                rec = smp.tile([1, 1], F32, tag="rec")
                nc.vector.reciprocal(rec[:], ssum[:])
                prob = smp.tile([1, S], F32, tag="prob")
                nc.vector.tensor_scalar_mul(prob[:], exp_sb[:], rec[:])
                nc.gpsimd.dma_start(out[bi:bi + 1, :], prob[:])

    nc.compile()
    return nc


_NC_CACHE = None


def _get_nc():
    global _NC_CACHE
    if _NC_CACHE is None:
        _NC_CACHE = build()
    return _NC_CACHE


def run(inputs, trace=False, trace_kwargs=None):
    hidden = np.ascontiguousarray(np.asarray(inputs["hidden"], dtype=np.float32))
    enc = np.ascontiguousarray(
        np.asarray(inputs["encoder_outputs"], dtype=np.float32)
    )
    W = np.ascontiguousarray(np.asarray(inputs["W"], dtype=np.float32))
    b = np.ascontiguousarray(np.asarray(inputs["b"], dtype=np.float32))
    v = np.ascontiguousarray(np.asarray(inputs["v"], dtype=np.float32))

    nc = _get_nc()
    in_maps = []
    for c in range(N_CORES):
        lo, hi = c * B_LOC, (c + 1) * B_LOC
        in_maps.append(
            {
                "hidden": hidden[lo:hi],
                "enc": enc[lo:hi],
                "W": W,
                "b": b,
                "v": v,
            }
        )
    res = bass_utils.run_bass_kernel_spmd(
        nc,
        in_maps,
        core_ids=list(range(N_CORES)),
        trace=trace,
        **(trace_kwargs or {}),
    )
    full = np.concatenate([res.results[c]["out"] for c in range(N_CORES)], axis=0)
    return full, res


def kernel(**inputs) -> np.ndarray:
    full, _ = run(inputs, trace=False)
    return full


# revision 15
# speedup vs baseline: 1.0281x; 1.0281x over previous
"""Bass/Tile TRN2 kernel for nn_Attention (Bahdanau-style attention scores).

Computation (per batch b):
    energy[s, h] = tanh( (enc[b] @ We)[s, h] + (hidden[b] @ Wh)[h] + bias[h] )
    scores[s]    = sum_h energy[s, h] * v[h]
    out[b]       = softmax(scores)

Sharding: data-parallel over batch B=32 across 8 cores (4 batches/core);
W, b, v replicated.

Per-core device program (all matmuls on PE):
  - enc tiles are PE-transposed (fp32, exact) to get the contraction dim (e)
    onto partitions.
  - main matmul We.T-tile @ encT in float32r (TF32-like, 1 cyc/row at N=512,
    fp32 PSUM accumulate), output layout [h, s] so the (h@Wh + b) bias is a
    per-partition scalar fused into the ScalarE tanh.
  - v-dot as a k=h matmul with v as a [128,1] stationary.
  - softmax over s on partition 0 (reduce_max -> exp with fused sum -> mul).
"""

import os
import ml_dtypes
import numpy as np

import concourse.bass as bass
import concourse.tile as tile
from concourse import bacc, mybir
from concourse import bass_utils
from concourse.masks import make_identity

F32 = mybir.dt.float32
F32R = mybir.dt.float32r
BF16 = mybir.dt.bfloat16
AFT = mybir.ActivationFunctionType
AXX = mybir.AxisListType.X

N_CORES = 8
B = 32
B_LOC = B // N_CORES  # 4
S = 1024
H = 512
E2 = 2 * H  # 1024
P = 128
N_HT = H // P   # 4 h-tiles
N_ET = E2 // P  # 8 e-tiles
N_SC = S // 512  # 2 s-chunks of 512


USE_BF16 = True


def build(use_bf16=None):
    if use_bf16 is None:
        use_bf16 = USE_BF16
    nc = bacc.Bacc("TRN2", target_bir_lowering=False, debug=False)
    hidden = nc.dram_tensor("hidden", [B_LOC, H], F32, kind="ExternalInput").ap()
    enc = nc.dram_tensor(
        "enc", [B_LOC, S, E2], BF16 if use_bf16 else F32, kind="ExternalInput"
    ).ap()
    W = nc.dram_tensor("W", [3 * H, H], F32, kind="ExternalInput").ap()
    bvec = nc.dram_tensor("b", [H], F32, kind="ExternalInput").ap()
    vvec = nc.dram_tensor("v", [H], F32, kind="ExternalInput").ap()
    out = nc.dram_tensor("out", [B_LOC, S], F32, kind="ExternalOutput").ap()

    with tile.TileContext(nc) as tc:
        with (
            tc.tile_pool(name="consts", bufs=1) as consts,
            tc.tile_pool(name="encp", bufs=8) as encp,
            tc.tile_pool(name="encTp", bufs=3 if USE_BF16 else 12) as encTp,
            tc.tile_pool(name="enccp", bufs=6) as enccp,
            tc.tile_pool(name="energyp", bufs=6) as energyp,
            tc.tile_pool(name="smp", bufs=2) as smp,
            tc.tile_pool(name="tpps", bufs=3, space="PSUM") as tpps,
            tc.tile_pool(name="outps", bufs=4, space="PSUM") as outps,
            tc.tile_pool(name="scps", bufs=1, space="PSUM") as scps,
        ):
            # ---- prefetch first s-chunk of enc on the sync (HWDGE) queue ----
            first_enc = None
            first_encT = None
            if use_bf16:
                first_encT = encTp.tile(
                    [P, N_ET, 512], BF16, tag="encT", name="encT_pre"
                )
                nc.sync.dma_start(first_encT[:], enc[0, 0:512, :], transpose=True)
            else:
                first_enc = []
                for st in range(4):
                    t0 = encp.tile([P, E2], F32, tag="enc", name=f"enc_pre{st}")
                    nc.sync.dma_start(t0[:], enc[0, st * P:(st + 1) * P, :])
                    first_enc.append(t0)

            # ---- constants: weight/vector DMAs on the gpsimd (SWDGE) queue ----
            ident = consts.tile([P, P], F32)
            make_identity(nc, ident[:])
            cast_dt = BF16 if use_bf16 else F32R
            ident_c = consts.tile([P, P], cast_dt)
            nc.vector.tensor_copy(ident_c[:], ident[:])

            We_sb = consts.tile([P, N_ET, H], F32)  # [e_in_tile, e_tile, h]
            We_r = consts.tile([P, N_ET, H], cast_dt)
            for j in range(N_ET):
                nc.gpsimd.dma_start(
                    We_sb[:, j, :], W[H + j * P:H + (j + 1) * P, :]
                )
                nc.vector.tensor_copy(We_r[:, j, :], We_sb[:, j, :])
            Wh_sb = consts.tile([P, N_HT, H], F32)
            nc.gpsimd.dma_start(
                Wh_sb[:], W[:H, :].rearrange("(j e) h -> e j h", e=P)
            )
            Wh_r = consts.tile([P, N_HT, H], F32R)
            nc.vector.tensor_copy(Wh_r[:], Wh_sb[:])
            v_sb = consts.tile([P, N_HT], F32)  # column i = v[i*128:(i+1)*128]
            nc.gpsimd.dma_start(v_sb[:], vvec.rearrange("(i e) -> e i", e=P))
            v_r = consts.tile([P, N_HT], F32R)
            nc.vector.tensor_copy(v_r[:], v_sb[:])
            b_sb = consts.tile([P, N_HT], F32)
            nc.gpsimd.dma_start(b_sb[:], bvec.rearrange("(i e) -> e i", e=P))
            hT_sb = consts.tile([P, N_HT, B_LOC], F32)  # hidden^T tiles [e, b]
            for t in range(N_HT):
                nc.gpsimd.dma_start(
                    hT_sb[:, t, :],
                    hidden[:, t * P:(t + 1) * P].rearrange("b e -> e b"),
                )
            hT_r = consts.tile([P, N_HT, B_LOC], F32R)
            nc.vector.tensor_copy(hT_r[:], hT_sb[:])

            bias_sb = consts.tile([P, N_HT, B_LOC], F32)

            def emit_bias_setup():
                # hproj as [b, h] wide-N matmul, then PE-transpose to [h, b];
                # bias[h, b] = hproj[h, b] + b[h]
                ps_h = tpps.tile([B_LOC, H], F32, tag="tstage", name="ps_h")
                for j in range(N_HT):
                    nc.tensor.matmul(
                        ps_h[:],
                        hT_r[:, j, :],
                        Wh_r[:, j, :],
                        start=(j == 0),
                        stop=(j == N_HT - 1),
                    )
                hp_sb = consts.tile([B_LOC, H], F32, name="hp_sb")
                nc.vector.tensor_copy(hp_sb[:], ps_h[:])
                for i in range(N_HT):
                    tp_i = tpps.tile([P, B_LOC], F32, tag="tstage", name=f"tp_i{i}")
                    nc.tensor.transpose(
                        tp_i[:], hp_sb[:, i * P:(i + 1) * P], ident[:B_LOC, :B_LOC]
                    )
                    nc.vector.tensor_scalar_add(
                        bias_sb[:, i, :], tp_i[:], b_sb[:, i:i + 1]
                    )

            # ---- main loop ----
            for bi in range(B_LOC):
                scores_sb = smp.tile([1, S], F32, tag="scores")
                for sc in range(N_SC):
                    s0 = sc * 512
                    psum_out = [
                        outps.tile([P, 512], F32, tag="mmout", name=f"mmout{i}")
                        for i in range(N_HT)
                    ]

                    if use_bf16:
                        # enc arrives bf16 in DRAM; the DMA xbar transposes a
                        # whole [512, 1024] s-chunk straight into SBUF as
                        # [e_p, e_tile, s] — no PE transposes, no casts.
                        if bi == 0 and sc == 0:
                            encT_all = first_encT
                        else:
                            encT_all = encTp.tile(
                                [P, N_ET, 512], BF16, tag="encT", name="encT_all"
                            )
                            nc.sync.dma_start(
                                encT_all[:],
                                enc[bi, s0:s0 + 512, :],
                                transpose=True,
                            )
                        for j in range(N_ET):
                            for i in range(N_HT):
                                nc.tensor.matmul(
                                    psum_out[i][:],
                                    We_r[:, j, i * P:(i + 1) * P],
                                    encT_all[:, j, :],
                                    start=(j == 0),
                                    stop=(j == N_ET - 1),
                                )
                    else:
                        if bi == 0 and sc == 0:
                            enc_tiles = first_enc
                        else:
                            enc_tiles = []
                            for st in range(4):
                                t = encp.tile([P, E2], F32, tag="enc")
                                nc.sync.dma_start(
                                    t[:], enc[bi, s0 + st * P: s0 + (st + 1) * P, :]
                                )
                                enc_tiles.append(t)

                        enc_c = []
                        for st in range(4):
                            ec = enccp.tile(
                                [P, E2], cast_dt, tag="encc", name=f"encc{st}"
                            )
                            nc.vector.tensor_copy(ec[:], enc_tiles[st][:])
                            enc_c.append(ec)

                        encT = [None] * N_ET

                        def emit_transpose(j):
                            tp = tpps.tile(
                                [P, 512], cast_dt, tag="tstage", name=f"tp{j}"
                            )
                            for st in range(4):
                                nc.tensor.transpose(
                                    tp[:, st * P:(st + 1) * P],
                                    enc_c[st][:, j * P:(j + 1) * P],
                                    ident_c[:],
                                )
                            e = encTp.tile(
                                [P, 512], cast_dt, tag="encT", name=f"encT{j}"
                            )
                            nc.vector.tensor_copy(e[:], tp[:])
                            encT[j] = e

                        def emit_matmuls(j):
                            for i in range(N_HT):
                                nc.tensor.matmul(
                                    psum_out[i][:],
                                    We_r[:, j, i * P:(i + 1) * P],
                                    encT[j][:],
                                    start=(j == 0),
                                    stop=(j == N_ET - 1),
                                )

                        # software-pipelined emission: transposes run 2 e-slices
                        # ahead of the matmuls so the PE never waits on the DVE
                        # PSUM->SBUF copy.
                        if bi == 0 and sc == 0:
                            for j in range(N_ET):
                                emit_transpose(j)
                            for j in range(N_ET):
                                emit_matmuls(j)
                        else:
                            emit_transpose(0)
                            emit_transpose(1)
                            for j in range(N_ET):
                                if j + 2 < N_ET:
                                    emit_transpose(j + 2)
                                emit_matmuls(j)

                    if bi == 0 and sc == 0:
                        # placed here so the slow weight-DMA -> cast chain it
                        # depends on never blocks the chunk-0 PE work (the PE
                        # executes strictly in program order).
                        emit_bias_setup()

                    sc_ps = scps.tile([1, 512], F32, tag="scores_ps")
                    for i in range(N_HT):
                        en = energyp.tile([P, 512], F32R, tag="energy", name=f"en{i}")
                        nc.scalar.activation(
                            en[:],
                            psum_out[i][:],
                            AFT.Tanh,
                            bias=bias_sb[:, i, bi:bi + 1],
                        )
                        nc.tensor.matmul(
                            sc_ps[:],
                            v_r[:, i:i + 1],
                            en[:],
                            start=(i == 0),
                            stop=(i == N_HT - 1),
                        )
                    nc.vector.tensor_copy(scores_sb[:, s0:s0 + 512], sc_ps[:])

                # ---- softmax over s (partition 0) ----
                negmax = smp.tile([1, 1], F32, tag="negmax")
                nc.vector.reduce_max(
                    out=negmax[:], in_=scores_sb[:], axis=AXX, negate=True
                )
                exp_sb = smp.tile([1, S], F32, tag="exp")
                ssum = smp.tile([1, 1], F32, tag="ssum")
                nc.scalar.activation(
                    exp_sb[:], scores_sb[:], AFT.Exp, bias=negmax[:], accum_out=ssum[:]
                )
                rec = smp.tile([1, 1], F32, tag="rec")
                nc.vector.reciprocal(rec[:], ssum[:])
                prob = smp.tile([1, S], F32, tag="prob")
                nc.vector.tensor_scalar_mul(prob[:], exp_sb[:], rec[:])
                nc.gpsimd.dma_start(out[bi:bi + 1, :], prob[:])

    nc.compile()
    return nc


_NC_CACHE = None


def _get_nc():
    global _NC_CACHE
    if _NC_CACHE is None:
        _NC_CACHE = build()
    return _NC_CACHE


def run(inputs, trace=False, trace_kwargs=None):
    hidden = np.ascontiguousarray(np.asarray(inputs["hidden"], dtype=np.float32))
    enc = np.ascontiguousarray(
        np.asarray(inputs["encoder_outputs"], dtype=np.float32)
    )
    if USE_BF16:
        enc = np.ascontiguousarray(enc.astype(ml_dtypes.bfloat16))
    W = np.ascontiguousarray(np.asarray(inputs["W"], dtype=np.float32))
    b = np.ascontiguousarray(np.asarray(inputs["b"], dtype=np.float32))
    v = np.ascontiguousarray(np.asarray(inputs["v"], dtype=np.float32))

    nc = _get_nc()
    in_maps = []
    for c in range(N_CORES):
        lo, hi = c * B_LOC, (c + 1) * B_LOC
        in_maps.append(
            {
                "hidden": hidden[lo:hi],
                "enc": enc[lo:hi],
                "W": W,
                "b": b,
                "v": v,
            }
        )
    res = bass_utils.run_bass_kernel_spmd(
        nc,
        in_maps,
        core_ids=list(range(N_CORES)),
        trace=trace,
        **(trace_kwargs or {}),
    )
    full = np.concatenate([res.results[c]["out"] for c in range(N_CORES)], axis=0)
    return full, res


def kernel(**inputs) -> np.ndarray:
    full, _ = run(inputs, trace=False)
    return full
